# revision 6
# baseline (speedup 1.0000x reference)
"""AccumulateNeighbours (meanmax) Trainium2 kernel.

out[v] = concat(mean_k feat[nidx[v,k]], max_k feat[nidx[v,k]])  -> [V, 2F]

Strategy: shard vertices (rows of feat/nidx) across the 8 NeuronCores; the
feat table is replicated into each core's HBM so the gather is fully local
and data-parallel. Per core, loop over 128-vertex tiles:
  1. DMA the tile's neighbour indices [128, K] into SBUF (HWDGE).
  2. K per-partition indirect DMAs (SWDGE) gather the neighbour rows into
     a [128, K, F] SBUF tile -- the HW DGE supports one dynamic offset per
     partition per instruction, so each instruction fetches 128 rows.
  3. Vector-engine reductions over K produce mean (via sum * 1/K on the
     scalar engine) and max; results land in a [128, 2F] tile.
  4. DMA the output tile back to HBM.
Tile pools double-buffer everything so gathers stream back-to-back; the
kernel is bound by SWDGE descriptor generation on the GpSimd Q7 engine.

int64 nidx is handled zero-copy: the little-endian low words are read with
a stride-2 int32 access pattern.
"""

import numpy as np

import concourse.bacc as bacc
import concourse.bass as bass
import concourse.mybir as mybir
import concourse.tile as tile
from concourse import bass_utils

V, K, F = 150000, 32, 96
NCORES = 8
VS = V // NCORES  # 18750 vertices per core
P = 128

GATHER_BUFS = 4
IDX_BUFS = 4
OUT_BUFS = 4

_prog_cache: dict = {}


def _build(idx_cols: int, idx_step: int):
    nc = bacc.Bacc("TRN2", target_bir_lowering=False, debug=False)
    feat_d = nc.dram_tensor("feat", [V, F], mybir.dt.float32, kind="ExternalInput")
    nidx_d = nc.dram_tensor("nidx", [VS, idx_cols], mybir.dt.int32, kind="ExternalInput")
    out_d = nc.dram_tensor("out", [VS, 2 * F], mybir.dt.float32, kind="ExternalOutput")

    feat_ap = feat_d.ap()
    nidx_ap = nidx_d.ap()
    out_ap = out_d.ap()

    ntiles = (VS + P - 1) // P
    with tile.TileContext(nc) as tc:
        with (
            tc.tile_pool(name="idx", bufs=IDX_BUFS) as idx_pool,
            tc.tile_pool(name="gather", bufs=GATHER_BUFS) as g_pool,
            tc.tile_pool(name="sum", bufs=2) as s_pool,
            tc.tile_pool(name="out", bufs=OUT_BUFS) as o_pool,
        ):
            for t in range(ntiles):
                rows = min(P, VS - t * P)
                idx_tile = idx_pool.tile([P, idx_cols], mybir.dt.int32)
                nc.sync.dma_start(
                    idx_tile[:rows, :], nidx_ap[t * P : t * P + rows, :]
                )
                g_tile = g_pool.tile([P, K, F], mybir.dt.float32)
                for j in range(K):
                    nc.gpsimd.indirect_dma_start(
                        out=g_tile[:rows, j],
                        out_offset=None,
                        in_=feat_ap[:, :],
                        in_offset=bass.IndirectOffsetOnAxis(
                            ap=idx_tile[:rows, j * idx_step : j * idx_step + 1],
                            axis=0,
                        ),
                    )
                perm = g_tile[:rows].rearrange("p k f -> p f k")
                sum_tile = s_pool.tile([P, F], mybir.dt.float32)
                nc.vector.reduce_sum(sum_tile[:rows], perm, axis=mybir.AxisListType.X)
                o_tile = o_pool.tile([P, 2 * F], mybir.dt.float32)
                nc.scalar.mul(o_tile[:rows, 0:F], sum_tile[:rows], 1.0 / K)
                nc.vector.reduce_max(
                    o_tile[:rows, F : 2 * F], perm, axis=mybir.AxisListType.X
                )
                nc.sync.dma_start(out_ap[t * P : t * P + rows, :], o_tile[:rows, :])
    nc.compile()
    return nc


def _get_prog(idx_cols, idx_step):
    key = (idx_cols, idx_step, GATHER_BUFS)
    if key not in _prog_cache:
        _prog_cache[key] = _build(idx_cols, idx_step)
    return _prog_cache[key]


def kernel(feat: np.ndarray, nidx: np.ndarray, **run_kwargs):
    assert feat.shape == (V, F), feat.shape
    assert nidx.shape == (V, K), nidx.shape
    feat = np.ascontiguousarray(feat, dtype=np.float32)
    if nidx.dtype == np.int64:
        nidx = np.ascontiguousarray(nidx)
        nidx32 = nidx.view(np.int32)  # [V, 2K]; low word = value (LE)
        idx_cols, idx_step = 2 * K, 2
    else:
        nidx32 = np.ascontiguousarray(nidx.astype(np.int32, copy=False))
        idx_cols, idx_step = K, 1

    nc = _get_prog(idx_cols, idx_step)
    in_maps = [
        {"feat": feat, "nidx": nidx32[c * VS : (c + 1) * VS]} for c in range(NCORES)
    ]
    res = bass_utils.run_bass_kernel_spmd(
        nc, in_maps, core_ids=list(range(NCORES)), **run_kwargs
    )
    out = np.concatenate([res.results[c]["out"] for c in range(NCORES)], axis=0)
    if run_kwargs:
        return out, res
    return out


# revision 7
# speedup vs baseline: 2.4931x; 2.4931x over previous
"""Hybrid AccumulateNeighbours: slots k=16..31 via per-partition indirect
gathers (SWDGE pair 0), slots k=0..15 via 5-segment sentinel dma_gather on
SWDGE queues 1-3 (concurrent Q7 pairs). Mean/max corrections as in kernel_b.
"""

import numpy as np

import concourse.bacc as bacc
import concourse.bass as bass
import concourse.mybir as mybir
import concourse.tile as tile
from concourse import bass_utils, library_config

P = 128
FP = 128
SENT = -16.0
K = 32
HK = 16            # B-side slots (k = 0..15)
NI = P * HK        # 2048
AK = K - HK        # A-side slots (k = 16..31)

V, F = 150000, 96
NSEG, SEGR = 5, 30000
NCORES = 8
VS = V // NCORES

_prog_cache: dict = {}


def _build(v, vs, nseg, segr):
    ntiles = (vs + P - 1) // P
    rows_seg = segr + 1

    nc = bacc.Bacc(
        "TRN2", target_bir_lowering=False, debug=False, num_swdge_queues=4
    )
    feat_d = nc.dram_tensor("feat", [v, F], mybir.dt.float32, kind="ExternalInput")
    featb_d = nc.dram_tensor(
        "featb", [nseg * rows_seg, FP], mybir.dt.float32, kind="ExternalInput"
    )
    nidxa_d = nc.dram_tensor("nidxa", [vs, AK], mybir.dt.int32, kind="ExternalInput")
    nw_d = nc.dram_tensor(
        "nw", [ntiles, P, NI // 16], mybir.dt.int32, kind="ExternalInput"
    )
    out_d = nc.dram_tensor("out", [vs, 2 * F], mybir.dt.float32, kind="ExternalOutput")
    feat_ap, featb_ap, out_ap = feat_d.ap(), featb_d.ap(), out_d.ap()

    corr = 16.0 * (nseg - 1) * HK / K  # sentinel de-bias on the mean: +32

    qctr = [0]

    with tile.TileContext(nc) as tc:
        with (
            tc.tile_pool(name="idxa", bufs=4) as ia_pool,
            tc.tile_pool(name="ga", bufs=4) as ga_pool,
            tc.tile_pool(name="idx32", bufs=3) as i32_pool,
            tc.tile_pool(name="prep", bufs=2) as prep_pool,
            tc.tile_pool(name="i16", bufs=3) as i16_pool,
            tc.tile_pool(name="bdst", bufs=2) as b_pool,
            tc.tile_pool(name="scr", bufs=2) as scr_pool,
            tc.tile_pool(name="part", bufs=3) as part_pool,
            tc.tile_pool(name="out", bufs=4) as o_pool,
        ):
            nc.gpsimd.load_library(library_config.mlp)
            mx = mybir.AluOpType.max
            ad = mybir.AluOpType.add

            for t in range(ntiles):
                rows = min(P, vs - t * P)

                # ---- B side: slots 0..15 via 5-segment dma_gather
                idx32 = i32_pool.tile([P, NI // 16], mybir.dt.int32)
                nc.sync.dma_start(idx32[:], nw_d.ap()[t])
                bdst = b_pool.tile([P, nseg, HK, FP], mybir.dt.float32)
                i16 = i16_pool.tile([P, nseg, NI // 16], mybir.dt.int16)
                for s in range(nseg):
                    t32 = prep_pool.tile([P, NI // 16], mybir.dt.int32)
                    nc.vector.tensor_scalar_add(
                        out=t32[:], in0=idx32[:], scalar1=-s * segr
                    )
                    nc.vector.tensor_scalar_min(
                        out=i16[:, s],
                        in0=t32[:].bitcast(mybir.dt.uint32),
                        scalar1=segr,
                    )
                    nc.gpsimd.dma_gather(
                        out_ap=bdst[:, s],
                        in_ap=featb_ap[s * rows_seg : (s + 1) * rows_seg, :],
                        idxs_ap=i16[:, s],
                        num_idxs=NI,
                        num_idxs_reg=NI,
                        elem_size=FP,
                        single_packet=False,
                        queue_num=1 + (qctr[0] % 3),
                    )
                    qctr[0] += 1

                # ---- A side: slots 16..31 via per-partition indirect
                idxa = ia_pool.tile([P, AK], mybir.dt.int32)
                nc.sync.dma_start(idxa[:rows, :], nidxa_d.ap()[t * P : t * P + rows])
                ga = ga_pool.tile([P, AK, F], mybir.dt.float32)
                for j in range(AK):
                    nc.gpsimd.indirect_dma_start(
                        out=ga[:rows, j],
                        out_offset=None,
                        in_=feat_ap[:, :],
                        in_offset=bass.IndirectOffsetOnAxis(
                            ap=idxa[:rows, j : j + 1], axis=0
                        ),
                    )

                # ---- B trees (80 chunks of FP)
                bv = bdst[:].rearrange("p s k f -> p (s k) f")
                scr = scr_pool.tile([P, (nseg * HK) // 2, FP], mybir.dt.float32)
                smax = part_pool.tile([P, FP], mybir.dt.float32, tag="pmax")
                ssum = part_pool.tile([P, FP], mybir.dt.float32, tag="psum")

                def fold(dst, a, b, op):
                    nc.vector.tensor_tensor(out=dst, in0=a, in1=b, op=op)

                fold(scr[:, 0:40], bv[:, 0:40], bv[:, 40:80], mx)
                fold(scr[:, 0:20], scr[:, 0:20], scr[:, 20:40], mx)
                fold(scr[:, 0:10], scr[:, 0:10], scr[:, 10:20], mx)
                fold(scr[:, 0:5], scr[:, 0:5], scr[:, 5:10], mx)
                fold(scr[:, 0:2], scr[:, 0:2], scr[:, 2:4], mx)
                fold(scr[:, 0:1], scr[:, 0:1], scr[:, 1:2], mx)
                fold(smax[:], scr[:, 0, :], scr[:, 4, :], mx)
                fold(bv[:, 0:40], bv[:, 0:40], bv[:, 40:80], ad)
                fold(bv[:, 0:20], bv[:, 0:20], bv[:, 20:40], ad)
                fold(bv[:, 0:10], bv[:, 0:10], bv[:, 10:20], ad)
                fold(bv[:, 0:5], bv[:, 0:5], bv[:, 5:10], ad)
                fold(bv[:, 0:2], bv[:, 0:2], bv[:, 2:4], ad)
                fold(bv[:, 0:1], bv[:, 0:1], bv[:, 1:2], ad)
                fold(ssum[:], bv[:, 0, :], bv[:, 4, :], ad)

                # ---- A reduces
                perm = ga[:rows].rearrange("p k f -> p f k")
                asum = part_pool.tile([P, F], mybir.dt.float32, tag="asum")
                amax = part_pool.tile([P, F], mybir.dt.float32, tag="amax")
                nc.vector.reduce_sum(asum[:rows], perm, axis=mybir.AxisListType.X)
                nc.vector.reduce_max(amax[:rows], perm, axis=mybir.AxisListType.X)

                # ---- combine
                o_tile = o_pool.tile([P, 2 * F], mybir.dt.float32)
                tsum = part_pool.tile([P, F], mybir.dt.float32, tag="tsum")
                fold(tsum[:rows], asum[:rows], ssum[:rows, :F], ad)
                nc.scalar.activation(
                    o_tile[:rows, 0:F],
                    tsum[:rows],
                    mybir.ActivationFunctionType.Copy,
                    bias=corr,
                    scale=1.0 / K,
                )
                fold(o_tile[:rows, F : 2 * F], amax[:rows], smax[:rows, :F], mx)
                nc.sync.dma_start(out_ap[t * P : t * P + rows, :], o_tile[:rows, :])
    nc.compile()
    return nc


def stage_featb(feat, nseg, segr):
    fb = np.zeros((nseg, segr + 1, FP), np.float32)
    fb[:, :segr, :F] = feat.reshape(nseg, segr, F)
    fb[:, segr, :] = SENT
    return fb.reshape(nseg * (segr + 1), FP)


def stage_nidx_wrap(nidx32, ncores, vs):
    """B-side wrap: slots k=0..HK-1 -> [ncores, ntiles, 128, NI//16]."""
    ntiles = (vs + P - 1) // P
    pad_vs = ntiles * P
    arr = np.zeros((ncores, pad_vs, HK), np.int32)
    arr[:, :vs] = nidx32.reshape(ncores, vs, K)[:, :, :HK]
    arr = arr.reshape(ncores, ntiles, P, HK)
    p_ar = np.arange(P)
    c_ar = np.arange(HK)
    w = np.zeros((ncores, ntiles, 16, NI // 16), np.int32)
    pp, cc = np.meshgrid(p_ar, c_ar, indexing="ij")
    w[:, :, pp % 16, cc * 8 + pp // 16] = arr.transpose(0, 1, 2, 3)
    w = np.tile(w, (1, 1, 8, 1))
    return np.ascontiguousarray(w)


def kernel(feat: np.ndarray, nidx: np.ndarray, **run_kwargs):
    assert feat.shape == (V, F)
    feat = np.ascontiguousarray(feat, dtype=np.float32)
    nidx32 = np.ascontiguousarray(nidx.astype(np.int32, copy=False))

    key = (V, VS, NSEG, SEGR)
    if key not in _prog_cache:
        _prog_cache[key] = _build(V, VS, NSEG, SEGR)
    nc = _prog_cache[key]

    featb = stage_featb(feat, NSEG, SEGR)
    nw = stage_nidx_wrap(nidx32, NCORES, VS)
    nidxa = np.ascontiguousarray(nidx32.reshape(NCORES, VS, K)[:, :, HK:])
    in_maps = [
        {"feat": feat, "featb": featb, "nidxa": nidxa[c], "nw": nw[c]}
        for c in range(NCORES)
    ]
    res = bass_utils.run_bass_kernel_spmd(
        nc, in_maps, core_ids=list(range(NCORES)), **run_kwargs
    )
    out = np.concatenate([res.results[c]["out"] for c in range(NCORES)], axis=0)
    if run_kwargs:
        return out, res
    return out
